# revision 17
# baseline (speedup 1.0000x reference)
"""Trainium2 Bass kernel for a 3-layer LSTM recurrent encoder.

Contract: kernel(**inputs) takes FULL inputs (as produced by
setup_inputs()) and returns the FULL output [256, 128, 16] fp32.

Strategy: data-parallel over batch (256 tracks -> 8 cores x 32), with the
recurrence computed in "orientation B": gates live in PSUM as
[128 gate-dims, 16 blocks x 32 tracks], i.e. gates transposed. Per 128-col
gate block g, step t:
    PS_t[:, g*32:(g+1)*32] = gxT_t  (DVE inject from SBUF staging)
                           += sum_k Wh[k,g-block]^T @ hT_{t-1}[k]
Stationary = Wh tile [128,128] bf16, moving = hT [128,32] bf16 -> measured
~34-60ns per matmul at saturation (vs 216ns for the A-orientation shape).
h comes out of the gate math already transposed ([h-dim, track]) so there
are NO PE transposes, and elementwise gate math runs on full 128-partition
tiles. The x-contribution (Wx part + bias) is computed 16 steps ahead in
batched 512-token matmuls (ACT epilogue applies the bias and casts to bf16
into an SBUF staging buffer); nothing round-trips through DRAM.

Layers: relu between layers is applied in-place on the h history two steps
behind the recurrence (after the last reader), so there is no batch relu
phase. Weights are double-buffered bf16; numerics: bf16 weights + bf16 h
gives ~6e-3 rel err (tolerance 2e-2).
"""

import sys

sys.path.insert(0, "/opt/trn_rl_repo")

import ml_dtypes
import numpy as np

import concourse.bacc as bacc
import concourse.mybir as mybir
import concourse.tile as tile
from concourse.bass_utils import run_bass_kernel_spmd

AF = mybir.ActivationFunctionType
F32 = mybir.dt.float32
BF16 = mybir.dt.bfloat16

B, T, F_IN = 256, 128, 16
H, L, OUT = 512, 3, 16
NCORES = 8
BL = B // NCORES          # 32 tracks per core
R = BL * T                # 4096 tokens per core
H4 = 4 * H                # 2048 gate columns
KT = H // 128             # 4 k-tiles of the hidden dim
NG = 16                   # gate blocks of 128 cols
SG = 16                   # steps per x-part staging group

_PROG = None


def _perm_b():
    """Permute gate cols from [i|g|f|o] (orig) to [i|f|o|g] block order."""
    return np.concatenate([
        np.arange(0, 512),          # i
        np.arange(1024, 1536),      # f
        np.arange(1536, 2048),      # o
        np.arange(512, 1024),       # g
    ])


def _build():
    nc = bacc.Bacc("TRN2", target_bir_lowering=False, debug=False,
                   num_devices=NCORES)

    xT_d = nc.dram_tensor("xT", [F_IN, R], BF16, kind="ExternalInput").ap()
    pw_d = nc.dram_tensor("pw", [F_IN, H], BF16, kind="ExternalInput").ap()
    pb_d = nc.dram_tensor("pb", [128, KT], F32, kind="ExternalInput").ap()
    wx_d = nc.dram_tensor("wx", [L, KT, 128, H4], BF16, kind="ExternalInput").ap()
    wh_d = nc.dram_tensor("wh", [L, KT, 128, H4], BF16, kind="ExternalInput").ap()
    bi_d = nc.dram_tensor("bi", [L, 128, NG], F32, kind="ExternalInput").ap()
    wo_d = nc.dram_tensor("wo", [KT, 128, OUT], BF16, kind="ExternalInput").ap()
    ob_d = nc.dram_tensor("ob", [OUT, 1], F32, kind="ExternalInput").ap()
    yT_d = nc.dram_tensor("yT", [OUT, R], F32, kind="ExternalOutput").ap()

    with tile.TileContext(nc) as tc:
        const = tc.alloc_tile_pool(name="const", bufs=1)
        z32b = const.tile([128, BL], BF16, tag="z32b")
        nc.vector.memset(z32b, 0.0)

        hA = tc.alloc_tile_pool(name="hA", bufs=1)
        hB = tc.alloc_tile_pool(name="hB", bufs=1)
        A = [hA.tile([128, R], BF16, tag=f"A{k}", name=f"A{k}") for k in range(KT)]
        Bt = [hB.tile([128, R], BF16, tag=f"B{k}", name=f"B{k}") for k in range(KT)]

        # weight slots (double buffered across layers)
        wts = tc.alloc_tile_pool(name="wts", bufs=1)
        wxt = [[wts.tile([128, H4], BF16, tag=f"wx{s}_{k}", name=f"wx{s}_{k}")
                for k in range(KT)] for s in range(2)]
        wht = [[wts.tile([128, H4], BF16, tag=f"wh{s}_{k}", name=f"wh{s}_{k}")
                for k in range(KT)] for s in range(2)]
        bit = [wts.tile([128, NG], F32, tag=f"bi{s}", name=f"bi{s}")
               for s in range(2)]
        # x-part staging buffers (one per 16-step group, ping-pong)
        gxt = [wts.tile([128, SG, 512], BF16, tag=f"gx{s}", name=f"gx{s}")
               for s in range(2)]

        def load_weights(l):
            s = l % 2
            for k in range(KT):
                nc.sync.dma_start(wxt[s][k][:], wx_d[l, k])
                nc.sync.dma_start(wht[s][k][:], wh_d[l, k])
            nc.sync.dma_start(bit[s][:], bi_d[l])

        load_weights(0)

        # ---- P0: projection -> A (bf16, relu) ----
        with tc.tile_pool(name="p0", bufs=1) as p0, \
             tc.tile_pool(name="p0ps", bufs=4, space="PSUM") as p0ps:
            xTt = p0.tile([F_IN, R], BF16, tag="xTt")
            nc.sync.dma_start(xTt[:], xT_d)
            pwt = p0.tile([F_IN, H], BF16, tag="pwt")
            nc.sync.dma_start(pwt[:], pw_d)
            pbt = p0.tile([128, KT], F32, tag="pbt")
            nc.sync.dma_start(pbt[:], pb_d)
            with nc.named_scope("P0"):
                for c in range(R // 512):
                    for k in range(KT):
                        ps = p0ps.tile([128, 512], F32)
                        nc.tensor.matmul(ps[:], pwt[:, k * 128:(k + 1) * 128],
                                         xTt[:, c * 512:(c + 1) * 512],
                                         start=True, stop=True)
                        nc.scalar.activation(A[k][:, c * 512:(c + 1) * 512],
                                             ps[:], AF.Relu,
                                             bias=pbt[:, k:k + 1])

        # ---- layers ----
        with tc.tile_pool(name="cs", bufs=2) as csp, \
             tc.tile_pool(name="sig", bufs=6) as sigp, \
             tc.tile_pool(name="tg", bufs=3) as tgp, \
             tc.tile_pool(name="tc_", bufs=3) as tcp, \
             tc.tile_pool(name="t12", bufs=4) as t12p, \
             tc.tile_pool(name="ps", bufs=1, space="PSUM") as psp, \
             tc.tile_pool(name="p1b", bufs=3, space="PSUM") as p1bp:

            # Two single-buffered gate tiles, one PSUM bank each. Separate
            # tiles so the Tile WAR tracking (tile-granular) never serializes
            # a gate-group's matmul writes against another group's ACT reads.
            # Single-buffered: step t+1's waves start only after h(t), by
            # which time step t's reads are long done.
            ps_if = psp.tile([128, 256], F32, tag="ps_if")  # i,f blocks 0..7
            ps_og = psp.tile([128, 256], F32, tag="ps_og")  # o,g blocks 8..15

            for l in range(L):
                s = l % 2
                if l + 1 < L:
                    load_weights(l + 1)

                c_sb = csp.tile([128, 128], F32, name="c_sb")
                nc.vector.memset(c_sb, 0.0)

                def relu_group(sg):
                    """Relu this layer's input tokens for staging group sg
                    (in place on A; off the per-step critical chain)."""
                    if l == 0:
                        return  # P0 already applied relu
                    tok0 = sg * 512
                    for k in range(KT):
                        nc.scalar.activation(A[k][:, tok0:tok0 + 512],
                                             A[k][:, tok0:tok0 + 512], AF.Relu)

                def p1b_one(g, sg):
                    """x-part for staging group sg (tokens sg*512..), block g.
                    Bias-add epilogue on DVE (it is idle late-step; ACT is
                    the chain-critical engine)."""
                    tok0 = sg * 512
                    ps = p1bp.tile([128, SG, 32], F32, name="xp")
                    for k in range(KT):
                        nc.tensor.matmul(
                            ps[:], wxt[s][k][:, g * 128:(g + 1) * 128],
                            A[k][:, tok0:tok0 + 512],
                            start=(k == 0), stop=(k == KT - 1))
                    nc.vector.tensor_scalar_add(
                        gxt[sg % 2][:, :, g * 32:(g + 1) * 32], ps[:],
                        bit[s][:, g:g + 1])

                with nc.named_scope(f"L{l}head"):
                    relu_group(0)
                    for g in range(NG):
                        p1b_one(g, 0)

                # injects on ACT: it has a ~1.3us idle window between the
                # sigmoids and tanh(c); on DVE they delay the t1/t2 chain
                def inject_if(t):
                    nc.scalar.copy(
                        ps_if[:], gxt[(t // SG) % 2][:, t % SG, 0:256])

                def inject_og(t):
                    nc.scalar.copy(
                        ps_og[:], gxt[(t // SG) % 2][:, t % SG, 256:512])

                inject_og(0)
                inject_if(0)

                with nc.named_scope(f"L{l}rec"):
                    for t in range(T):

                        def wave(group):
                            # full K accumulation for one gate-group; og
                            # first so tg/so fire while if still streams
                            tile, j0 = ((ps_og, 8) if group == "og"
                                        else (ps_if, 0))
                            for k in range(KT):
                                hsrc = (z32b[:] if t == 0
                                        else Bt[k][:, (t - 1) * BL:t * BL])
                                stop = k == KT - 1
                                for j in range(j0, j0 + 8):
                                    jo = (j - j0) * 32
                                    nc.tensor.matmul(
                                        tile[:, jo:jo + 32],
                                        wht[s][k][:, j * 128:(j + 1) * 128],
                                        hsrc, start=False, stop=stop)

                        wave("og")
                        wave("if")
                        boundary = t + 1 < T and (t + 1) % SG == 0
                        # gate math; each PSUM gate tile has its readers
                        # emitted before the next step's inject overwrites it
                        tg = tgp.tile([128, 128], F32)
                        nc.scalar.activation(tg[:], ps_og[:, 128:256], AF.Tanh)
                        so = sigp.tile([128, 128], F32)
                        nc.scalar.activation(so[:], ps_og[:, 0:128], AF.Sigmoid)
                        sif = sigp.tile([128, 256], F32)
                        nc.scalar.activation(sif[:], ps_if[:], AF.Sigmoid)
                        # injects after sif: program-order tie-break keeps
                        # the ACT queue clear for the chain-critical sigmoid
                        if t + 1 < T and not boundary:
                            inject_og(t + 1)
                            inject_if(t + 1)
                        t1 = t12p.tile([128, 128], F32)
                        nc.vector.tensor_mul(t1[:], sif[:, 0:128], tg[:])
                        t2 = t12p.tile([128, 128], F32)
                        nc.gpsimd.tensor_mul(t2[:], sif[:, 128:256], c_sb[:])
                        nc.vector.tensor_add(c_sb[:], t1[:], t2[:])
                        tc_ = tcp.tile([128, 128], F32)
                        nc.scalar.activation(tc_[:], c_sb[:], AF.Tanh)
                        # h blocks 0,1 on DVE (wave k0/k1 need them first),
                        # 2,3 on the otherwise-idle GPSIMD
                        for k in range(2):
                            nc.vector.tensor_mul(
                                Bt[k][:, t * BL:(t + 1) * BL],
                                so[:, k * 32:(k + 1) * 32],
                                tc_[:, k * 32:(k + 1) * 32])
                        for k in range(2, KT):
                            nc.gpsimd.tensor_mul(
                                Bt[k][:, t * BL:(t + 1) * BL],
                                so[:, k * 32:(k + 1) * 32],
                                tc_[:, k * 32:(k + 1) * 32])
                        # x-part for steps 16 ahead (4 matmuls + 1 epilogue
                        # per step fills PE while it waits on the h chain).
                        # relu after the gate math so the scheduler's
                        # program-order tie-break favors the chain ops.
                        if t < T - SG:
                            if t % SG == 0:
                                relu_group(t // SG + 1)
                            p1b_one(t % SG, t // SG + 1)
                        if boundary:
                            inject_og(t + 1)
                            inject_if(t + 1)

                A, Bt = Bt, A

        # ---- P2: output projection ----
        with tc.tile_pool(name="p2", bufs=1) as p2, \
             tc.tile_pool(name="p2s", bufs=4) as p2s, \
             tc.tile_pool(name="p2ps", bufs=4, space="PSUM") as p2ps:
            wo = []
            for k in range(KT):
                w = p2.tile([128, OUT], BF16, tag=f"wo{k}", name=f"wot{k}")
                nc.sync.dma_start(w[:], wo_d[k])
                wo.append(w)
            obt = p2.tile([OUT, 1], F32, tag="obt")
            nc.sync.dma_start(obt[:], ob_d)
            with nc.named_scope("P2"):
                for c in range(R // 512):
                    ps = p2ps.tile([OUT, 512], F32)
                    for k in range(KT):
                        nc.tensor.matmul(ps[:], wo[k][:],
                                         A[k][:, c * 512:(c + 1) * 512],
                                         start=(k == 0), stop=(k == KT - 1))
                    y = p2s.tile([OUT, 512], F32)
                    nc.scalar.activation(y[:], ps[:], AF.Identity, bias=obt[:])
                    nc.sync.dma_start(yT_d[:, c * 512:(c + 1) * 512], y[:])

        for p in (wts, hB, hA, const):
            p.release()

    nc.compile()
    return nc


def _get_prog():
    global _PROG
    if _PROG is None:
        _PROG = _build()
    return _PROG


def _stage_inputs(x, proj_w, proj_b, lstm_w, lstm_b, out_w, out_b):
    perm = _perm_b()
    bf = ml_dtypes.bfloat16
    lw = np.asarray(lstm_w, np.float32)
    lb = np.asarray(lstm_b, np.float32).copy()
    lb[:, 2 * H:3 * H] += 1.0  # forget-gate +1.0 folded into bias
    shared = {
        "pw": np.ascontiguousarray(np.asarray(proj_w, np.float32)).astype(bf),
        "pb": np.ascontiguousarray(
            np.asarray(proj_b, np.float32).reshape(KT, 128).T),
        "wx": np.ascontiguousarray(
            lw[:, :H, :][:, :, perm].reshape(L, KT, 128, H4)).astype(bf),
        "wh": np.ascontiguousarray(
            lw[:, H:, :][:, :, perm].reshape(L, KT, 128, H4)).astype(bf),
        "bi": np.ascontiguousarray(
            lb[:, perm].reshape(L, NG, 128).transpose(0, 2, 1)),
        "wo": np.ascontiguousarray(
            np.asarray(out_w, np.float32).reshape(KT, 128, OUT)).astype(bf),
        "ob": np.ascontiguousarray(
            np.asarray(out_b, np.float32).reshape(OUT, 1)),
    }
    x = np.asarray(x, np.float32)
    in_maps = []
    for c in range(NCORES):
        xs = x[c * BL:(c + 1) * BL]                     # [32, 128, 16]
        xT = np.ascontiguousarray(
            xs.transpose(2, 1, 0).reshape(F_IN, R)).astype(bf)
        in_maps.append({"xT": xT, **shared})
    return in_maps


def kernel(x, proj_w, proj_b, lstm_w, lstm_b, out_w, out_b, _trace=False):
    nc = _get_prog()
    in_maps = _stage_inputs(x, proj_w, proj_b, lstm_w, lstm_b, out_w, out_b)
    res = run_bass_kernel_spmd(nc, in_maps, core_ids=list(range(NCORES)),
                               trace=_trace)
    y = np.empty((B, T, OUT), np.float32)
    for c in range(NCORES):
        yT = res.results[c]["yT"]                       # [16, 4096]
        y[c * BL:(c + 1) * BL] = yT.reshape(OUT, T, BL).transpose(2, 1, 0)
    kernel._last_results = res
    return y
